# revision 3
# baseline (speedup 1.0000x reference)
"""Batched single-qubit gate application on 8 TRN2 NeuronCores — fp16 I/O.

Problem: state (B=2048, N=8192) complex (separate f32 re/im planes), apply a
2x2 complex gate G on qubit 5 (pairs at stride R=128 within 256-blocks):
    out[b, l, c, r] = sum_a state[b, l, a, r] * G[a, c],  l<32, r<128.
Returns stacked (2, B, N) f32 [re, im].

The correctness gate is rel_err < 2e-2; fp16 rounding of inputs, gate and
outputs gives ~9e-4, so all HBM traffic is fp16 — half the bytes of the f32
version (memory-bound target: 8.4 MB in + 8.4 MB out per core).

Sharding: pure data parallel over the batch dim, 256 rows/core. The host
interleaves re/im at row granularity (and casts to fp16) into one
[256, 2, 8192] tensor per core so every DMA touches all 128 SBUF partitions
with a 2-dim DRAM access pattern.

Per-core strategy: TensorE does ALL the work (fp16 matmul = 1 cycle/row, 4x
the f32 rate, so one engine now covers the whole element stream at ~58%
busy). The moving operand keeps the natural interleaved row layout
([re_b; im_b] pairs on partitions); stationary 128x128 matrices
W(a,c) = kron(I64, [[gr, gi], [-gi, gr]]) (host-built fp16, passed as an
input) turn each matmul into "complex-scale 64 rows by G[a,c]" with PSUM
accumulating the two a-terms in f32.

Chunks of [128 rows x 4096 elems] (1 MB DMAs); each chunk is two PSUM
phases of 2048 f32 (4 banks, double-buffered across the 8 banks). Per phase
ScalarE (ACT) evacuates the ls=0 half and VectorE (DVE) the ls=1 half,
casting f32 PSUM -> fp16 staging. Input DMAs ride the SP HWDGE ring (sync),
output DMAs ride SWDGE (gpsimd), so no engine both computes and issues the
heavy DMA stream.

reps>1 builds the same pipeline repeated back-to-back in one NEFF (sems keep
counting) — used only for steady-state hardware timing measurements.
"""

import sys

sys.path.insert(0, "/opt/trn_rl_repo")

from contextlib import ExitStack

import numpy as np

import concourse.bass as bass
import concourse.mybir as mybir
from concourse.bass_utils import run_bass_kernel_spmd

F16 = mybir.dt.float16
F32 = mybir.dt.float32

NCORES = 8
B = 2048
N = 8192
BC = B // NCORES  # 256 rows per core
JC = 8192  # j-chunk (fp16 elems per partition line)
NJ = N // JC  # 1
R = 128

_NC_CACHE = None


def _build_program(reps=1):
    nc = bass.Bass()

    sri = nc.declare_dram_parameter("sri", [BC, 2, N], F16, isOutput=False)
    wall = nc.declare_dram_parameter("wall", [128, 4, 128], F16, isOutput=False)
    opk = nc.declare_dram_parameter("opk", [BC, 2, N], F16, isOutput=True)

    # SBUF
    wsb = nc.alloc_sbuf_tensor("wsb", [128, 4, 128], F16)
    inP = [nc.alloc_sbuf_tensor(f"inP{s}", [128, JC], F16) for s in range(2)]
    stgP = [nc.alloc_sbuf_tensor(f"stgP{s}", [128, JC], F16) for s in range(2)]
    # PSUM: 4 tensors x 2 banks = 8 banks; phase ph uses set ph (ph in 0,1).
    # psp[2*ph + ls][:, c*512:(c+1)*512] is the (ph, ls, c) matmul target.
    psp = [nc.alloc_psum_tensor(f"ps{i}", [128, 1024], F32) for i in range(4)]

    K = 4 * reps  # chunks per program
    P = 4 * K  # PSUM phases

    # chunk free-axis lattice: [128, ph, ls, l, a|c, r]
    def lat_in(t, ph, ls, a):
        return t[:].rearrange(
            "p (ph ls l a r) -> p ph ls l a r", ph=4, ls=2, l=4, a=2, r=R
        )[:, ph, ls, :, a, :]

    def lat_stg(t, ph, ls):
        return t[:].rearrange(
            "p (ph ls l c r) -> p ph ls l c r", ph=4, ls=2, l=4, c=2, r=R
        )[:, ph, ls, :, :, :]

    def lat_ps(i):
        return psp[i][:].rearrange("p (c l r) -> p l c r", c=2, r=R)

    with ExitStack() as _ctx:
        block = _ctx.enter_context(nc.Block())
        sem = {
            n: _ctx.enter_context(nc.semaphore(n))
            for n in ["wS", "iP0", "iP1", "mmS", "cA", "cV", "oP0", "oP1"]
        }
        wS, mmS, cA, cV = sem["wS"], sem["mmS"], sem["cA"], sem["cV"]
        iP = [sem["iP0"], sem["iP1"]]
        oP = [sem["oP0"], sem["oP1"]]

        sri_flat = sri[:].rearrange("b e j -> (b e) j")
        opk_flat = opk[:].rearrange("b e j -> (b e) j")

        def rows(k):
            # 128 interleaved (row, re/im) DRAM rows of the flat [(BC 2), N] view
            g = k % 4
            return slice(128 * g, 128 * g + 128)

        def J(k):
            return slice(0, JC)

        @block.sync
        def _(sync):
            def issue_in(k):
                s = k & 1
                sync.dma_start(out=inP[s][:], in_=sri_flat[rows(k), J(k)]).then_inc(
                    iP[s], 16
                )

            issue_in(0)
            issue_in(1)
            for k in range(2, K):
                # inP[k&1] free once PE consumed chunk k-2 (its 4 phases done)
                sync.wait_ge(mmS, 4 * k - 4)
                issue_in(k)
            # final quiesce: every output DMA landed
            sync.wait_ge(oP[0], 16 * ((K + 1) >> 1))
            sync.wait_ge(oP[1], 16 * (K >> 1))

        @block.tensor
        def _(tensor):
            tensor.wait_ge(wS, 16)
            for p in range(P):
                k, ph = p >> 2, p & 3
                s = k & 1
                if ph == 0:
                    tensor.wait_ge(iP[s], 16 * ((k >> 1) + 1))
                if p >= 2:
                    # PSUM set p&1 free once phase p-2's copies are done
                    tensor.wait_ge(cA, p - 1)
                    tensor.wait_ge(cV, p - 1)
                last = None
                for c in range(2):
                    for a in range(2):
                        for ls in range(2):
                            last = tensor.matmul(
                                psp[2 * (ph & 1) + ls][:, c * 512 : (c + 1) * 512],
                                wsb[:, a * 2 + c, :],
                                lat_in(inP[s], ph, ls, a),
                                start=(a == 0),
                                stop=(a == 1),
                            )
                assert last is not None
                last.then_inc(mmS, 1)

        def evac(engine, copy, ls, sem_c):
            # per-phase PSUM->staging evacuation for one ls half
            if ls == 0:
                engine.dma_start(out=wsb[:], in_=wall[:]).then_inc(wS, 16)
            for p in range(P):
                k, ph = p >> 2, p & 3
                s = k & 1
                engine.wait_ge(mmS, p + 1)
                if ph == 0 and k >= 2:
                    # stgP[s] free once chunk k-2's output DMA completed
                    engine.wait_ge(oP[s], 16 * (k >> 1))
                copy(
                    lat_stg(stgP[s], ph, ls), lat_ps(2 * (ph & 1) + ls)
                ).then_inc(sem_c, 1)

        @block.scalar
        def _(scalar):
            evac(scalar, scalar.copy, 0, cA)

        @block.vector
        def _(vector):
            evac(vector, vector.tensor_copy, 1, cV)

        @block.gpsimd
        def _(gpsimd):
            for k in range(K):
                s = k & 1
                gpsimd.wait_ge(cA, 4 * k + 4)
                gpsimd.wait_ge(cV, 4 * k + 4)
                gpsimd.dma_start(
                    out=opk_flat[rows(k), J(k)], in_=stgP[s][:]
                ).then_inc(oP[s], 16)

    return nc


def _get_nc():
    global _NC_CACHE
    if _NC_CACHE is None:
        _NC_CACHE = _build_program()
    return _NC_CACHE


def _host_tensors(gate_real, gate_imag):
    gr = np.asarray(gate_real, dtype=np.float32)
    gi = np.asarray(gate_imag, dtype=np.float32)
    I64 = np.eye(64, dtype=np.float32)
    ws = []
    for a in range(2):
        for c in range(2):
            g2 = np.array(
                [[gr[a, c], gi[a, c]], [-gi[a, c], gr[a, c]]], dtype=np.float32
            )
            ws.append(np.kron(I64, g2))
    wall = np.stack(ws, axis=1).astype(np.float16)  # [128 k, 4 g, 128 m]
    return np.ascontiguousarray(wall)


def _in_maps(state_real, state_imag, wall):
    sr = np.asarray(state_real, dtype=np.float16)
    si = np.asarray(state_imag, dtype=np.float16)
    maps = []
    for i in range(NCORES):
        rows = slice(i * BC, (i + 1) * BC)
        sri = np.stack([sr[rows], si[rows]], axis=1)
        maps.append({"sri": sri, "wall": wall})
    return maps


def kernel(state_real, state_imag, gate_real, gate_imag):
    wall = _host_tensors(gate_real, gate_imag)

    nc = _get_nc()
    res = run_bass_kernel_spmd(
        nc, _in_maps(state_real, state_imag, wall), list(range(NCORES))
    )

    out = np.empty((2, B, N), dtype=np.float32)
    for i in range(NCORES):
        rows = slice(i * BC, (i + 1) * BC)
        opk = res.results[i]["opk"]  # [BC, 2, N] fp16
        out[0, rows] = opk[:, 0]
        out[1, rows] = opk[:, 1]
    return out


# revision 4
# speedup vs baseline: 1.7237x; 1.7237x over previous
"""Batched single-qubit gate application on 8 TRN2 NeuronCores — int8 I/O.

Problem: state (B=2048, N=8192) complex (separate f32 re/im planes), apply a
2x2 complex gate G on qubit 5: out[b,l,c,r] = sum_a state[b,l,a,r] * G[a,c].
Returns stacked (2, B, N) f32.

The correctness gate is rel_err < 2e-2. Inputs are quantized on the host to
int8 (scale s_in = max|plane|/127) and outputs are written as int8 with
per-output-column scales s_out[m] derived from a deterministic bound
(sum_k |M4[k,m]| * 127 * s_in), giving ~1.2e-2 exact worst-case rel err.
HBM traffic drops to 4.2 MB in + 4.2 MB out per core per rep; the binding
limit becomes the SBUF AXI fabric (~435 GB/s) because the input DMA casts
int8 -> fp16 in flight (SWDGE), writing 2x bytes into SBUF.

Layout trick: the host pre-permutes the state so SBUF partition p = 4b+2e+a
(b = row-in-32-group, e = re/im, a = qubit-5 bit) and the free axis is
(grp, l, r). The stationary matrix is then a single
kron(I32, M4) with M4[(e,a),(e',c)] the 4x4 real form of the complex 2x2
gate (x s_in/s_out[m] folded in), so ONE matmul pass computes every output
(no PSUM accumulation, no weight swaps, unit-stride moving operands). PSUM
holds int8-ready values; ACT/DVE alternate phases casting f32 PSUM -> int8
staging (round-to-nearest, saturating). GPSIMD issues the casting input
DMAs (SWDGE-only feature); sync issues output DMAs and quiesces. The host
dequantizes/unpermutes the int8 result (outside the NEFF).

reps>1 builds the same pipeline repeated back-to-back in one NEFF (sems keep
counting) — used only for steady-state hardware timing measurements.
"""

import sys

sys.path.insert(0, "/opt/trn_rl_repo")

from contextlib import ExitStack

import numpy as np

import concourse.bass as bass
import concourse.mybir as mybir
from concourse.bass_utils import run_bass_kernel_spmd

F16 = mybir.dt.float16
F32 = mybir.dt.float32
I8 = mybir.dt.int8

NCORES = 8
B = 2048
N = 8192
BC = B // NCORES  # 256 state rows per core
NB = 4  # blocks per rep (64 state rows each)
JB = 8192  # free elems per partition per block: (grp 2, l 32, r 128)
NPH = 8  # PSUM phases per block (1024 f32 each)

_NC_CACHE = None


def _build_program(reps=1):
    nc = bass.Bass()

    sq = nc.declare_dram_parameter("sq", [NB, 128, JB], I8, isOutput=False)
    wall = nc.declare_dram_parameter("wall", [128, 128], F16, isOutput=False)
    oq = nc.declare_dram_parameter("oq", [NB, 128, JB], I8, isOutput=True)

    wsb = nc.alloc_sbuf_tensor("wsb", [128, 128], F16)
    inT = [nc.alloc_sbuf_tensor(f"inT{s}", [128, JB], F16) for s in range(2)]
    stgT = [nc.alloc_sbuf_tensor(f"stgT{s}", [128, JB], I8) for s in range(2)]
    # 4 PSUM tensors x 2 banks = 8 banks; phase qg uses psp[qg % 4].
    psp = [nc.alloc_psum_tensor(f"ps{i}", [128, 1024], F32) for i in range(4)]

    T = NB * reps  # blocks per program
    PH = NPH * T  # global phases

    with ExitStack() as _ctx:
        block = _ctx.enter_context(nc.Block())
        sem = {
            n: _ctx.enter_context(nc.semaphore(n))
            for n in ["wS", "iP0", "iP1", "mmS", "cA", "cV", "oP0", "oP1"]
        }
        wS, mmS, cA, cV = sem["wS"], sem["mmS"], sem["cA"], sem["cV"]
        iP = [sem["iP0"], sem["iP1"]]
        oP = [sem["oP0"], sem["oP1"]]

        @block.gpsimd
        def _(gpsimd):
            def issue_in(t):
                # int8 HBM -> fp16 SBUF, cast in flight (SWDGE only)
                gpsimd.dma_start(out=inT[t & 1][:], in_=sq[t % NB]).then_inc(
                    iP[t & 1], 16
                )

            issue_in(0)
            if T > 1:
                issue_in(1)
            for t in range(2, T):
                # inT[t&1] free once PE consumed block t-2 (its 8 phases done)
                gpsimd.wait_ge(mmS, NPH * t - NPH)
                issue_in(t)

        @block.tensor
        def _(tensor):
            tensor.wait_ge(wS, 16)
            for qg in range(PH):
                t, q = qg >> 3, qg & 7
                if q == 0:
                    tensor.wait_ge(iP[t & 1], 16 * ((t >> 1) + 1))
                if qg >= 4:
                    # psp[qg%4] free once phase qg-4's evac copy completed
                    pq = qg - 4
                    tensor.wait_ge(cA if pq % 2 == 0 else cV, pq // 2 + 1)
                last = None
                for h in range(2):
                    c0 = 1024 * q + 512 * h
                    last = tensor.matmul(
                        psp[qg % 4][:, h * 512 : (h + 1) * 512],
                        wsb[:],
                        inT[t & 1][:, c0 : c0 + 512],
                        start=True,
                        stop=True,
                    )
                assert last is not None
                last.then_inc(mmS, 1)

        def evac(engine, copy, par, sem_c):
            # phases with qg%2==par, casting f32 PSUM -> int8 staging
            for qg in range(par, PH, 2):
                t, q = qg >> 3, qg & 7
                engine.wait_ge(mmS, qg + 1)
                if q == par and t >= 2:
                    # stgT[t&1] free once block t-2's output DMA completed
                    engine.wait_ge(oP[t & 1], 16 * (t >> 1))
                copy(
                    stgT[t & 1][:, 1024 * q : 1024 * (q + 1)], psp[qg % 4][:]
                ).then_inc(sem_c, 1)

        @block.scalar
        def _(scalar):
            scalar.dma_start(out=wsb[:], in_=wall[:]).then_inc(wS, 16)
            evac(scalar, scalar.copy, 0, cA)

        @block.vector
        def _(vector):
            evac(vector, vector.tensor_copy, 1, cV)

        @block.sync
        def _(sync):
            for t in range(T):
                sync.wait_ge(cA, 4 * (t + 1))
                sync.wait_ge(cV, 4 * (t + 1))
                sync.dma_start(out=oq[t % NB], in_=stgT[t & 1][:]).then_inc(
                    oP[t & 1], 16
                )
            sync.wait_ge(oP[0], 16 * ((T + 1) >> 1))
            sync.wait_ge(oP[1], 16 * (T >> 1))

    return nc


def _get_nc():
    global _NC_CACHE
    if _NC_CACHE is None:
        _NC_CACHE = _build_program()
    return _NC_CACHE


def _m4(gate_real, gate_imag):
    gr = np.asarray(gate_real, dtype=np.float64)
    gi = np.asarray(gate_imag, dtype=np.float64)
    M4 = np.zeros((4, 4))
    for a in range(2):
        for c in range(2):
            M4[0 * 2 + a, 0 * 2 + c] = gr[a, c]
            M4[1 * 2 + a, 0 * 2 + c] = -gi[a, c]
            M4[0 * 2 + a, 1 * 2 + c] = gi[a, c]
            M4[1 * 2 + a, 1 * 2 + c] = gr[a, c]
    return M4


def _host_tensors(gate_real, gate_imag, amax):
    M4 = _m4(gate_real, gate_imag)
    s_in = amax / 127.0
    s_out = np.abs(M4).sum(axis=0) * amax / 126.0  # per m=(e',c)
    M4s = M4 * (s_in / s_out[None, :])
    wall = np.kron(np.eye(32), M4s).astype(np.float16)  # [128 k, 128 m]
    return np.ascontiguousarray(wall), s_in, s_out


def _in_maps(state_real, state_imag, wall, s_in):
    sr = np.asarray(state_real, dtype=np.float32)
    si = np.asarray(state_imag, dtype=np.float32)
    maps = []
    for i in range(NCORES):
        rows = slice(i * BC, (i + 1) * BC)
        pl = np.stack([sr[rows], si[rows]], axis=1)  # [256, 2, 8192]
        q = np.clip(np.round(pl / s_in), -127, 127).astype(np.int8)
        # [256,2,8192] -> (t, grp, b, e, l, a, r) -> (t, b, e, a, grp, l, r)
        v = q.reshape(NB, 2, 32, 2, 32, 2, 128)
        v = v.transpose(0, 2, 3, 5, 1, 4, 6).reshape(NB, 128, JB)
        maps.append({"sq": np.ascontiguousarray(v), "wall": wall})
    return maps


def kernel(state_real, state_imag, gate_real, gate_imag):
    sr = np.asarray(state_real, dtype=np.float32)
    si = np.asarray(state_imag, dtype=np.float32)
    amax = float(max(np.abs(sr).max(), np.abs(si).max(), 1e-30))
    wall, s_in, s_out = _host_tensors(gate_real, gate_imag, amax)

    nc = _get_nc()
    res = run_bass_kernel_spmd(
        nc, _in_maps(sr, si, wall, s_in), list(range(NCORES))
    )

    out = np.empty((2, B, N), dtype=np.float32)
    sc = s_out.astype(np.float32)  # index m%4 = 2e'+c
    for i in range(NCORES):
        rows = slice(i * BC, (i + 1) * BC)
        oqv = res.results[i]["oq"]  # [NB, 128, JB] int8
        # partitions = (b, e', c); free = (grp, l, r)
        v = oqv.reshape(NB, 32, 2, 2, 2, 32, 128).astype(np.float32)
        v *= sc.reshape(1, 1, 2, 2, 1, 1, 1)
        # (t, b, e', c, grp, l, r) -> (e', t, grp, b, l, c, r)
        v = v.transpose(2, 0, 4, 1, 5, 3, 6).reshape(2, BC, N)
        out[0, rows] = v[0]
        out[1, rows] = v[1]
    return out
